# revision 1
# baseline (speedup 1.0000x reference)
"""Trainium2 Bass kernel for the per-cell-MLP "MAR one-sided missingness" model.

Model (per cell (n,t) of a 1024x128 grid):
    xc     = X[n, col_idx[n,t]]
    h      = relu(W_in[n,t,:,0]*xc + W_in[n,t,:,1]*X[n,t] + b_in[n,t,:])   # [H]
    out    = sigmoid(dot(W_out[n,t,:], h) + b_out[n,t])

Sharding: rows N split across 8 cores (128 rows each), fully data parallel.

Per-core layout: partition dim = t (128 cells of one row), free dim = h.
The neighbor gather X[n, col_idx[n,t]] runs on the PE as a one-hot matmul:
one-hot masks (a host-staged re-encoding of col_idx) are fp8 stationaries;
X rides as an f16 hi/lo split (lo pre-scaled by 2^12 to avoid denormals) so
the gathered values are exact to ~2.5e-7 relative.

Weights stream as four contiguous [t, n, h] tensors (w0, w1, b, wo).
Per 16-row superblock, software-pipelined one block deep (the back half of
block s is emitted alongside the front half of block s+1 so engines fill
their cross-engine waits with the next block's independent work):
  front(s):
    DMA  : w1, b, w0, wo, one-hot slices
    PE   : xc2[:, 2g:2g+2] = oh_g^T @ [Xhi | Xlo*2^12][:, n]   (per row)
    ACT  : xc2 copy; DVE: xc = (lo * 2^-12) + hi
    DVE  : m1 = w1 * broadcast(x)            (batched TT, stride-0 AP)
    Pool : v  = m1 + b                       (batched)
    ACT  : a0_g = w0_g * xc_g                (per row, per-partition scale)
  back(s) emitted with front(s+1):
    Pool : u  = a0 + v
    DVE  : r  = (u max 0) * wo               (batched STT)
    DVE  : red[:, g] = sum_h r               (batched reduce)
Epilogue: out = sigmoid(red + b_out^T), DMA out, host transposes back.

HBM-bandwidth bound: streams ~34 MB per core.
"""

import ml_dtypes
import numpy as np

N, T, H = 1024, 128, 128
M = 8            # cores
NR = N // M      # rows per core
G = 16
S = NR // G
LO_SCALE = float(2 ** 12)

_cache = {}


def _build():
    if "nc" in _cache:
        return _cache["nc"]
    import concourse.bacc as bacc
    import concourse.mybir as mybir
    import concourse.tile as tile

    f32 = mybir.dt.float32
    f16 = mybir.dt.float16
    f8 = mybir.dt.float8e4
    Alu = mybir.AluOpType
    Act = mybir.ActivationFunctionType

    nc = bacc.Bacc()
    w0all = nc.declare_dram_parameter("w0all", [T, NR, H], f32, isOutput=False)
    w1all = nc.declare_dram_parameter("w1all", [T, NR, H], f32, isOutput=False)
    ball = nc.declare_dram_parameter("ball", [T, NR, H], f32, isOutput=False)
    woall = nc.declare_dram_parameter("woall", [T, NR, H], f32, isOutput=False)
    ohall = nc.declare_dram_parameter("ohall", [128, NR * T], f8, isOutput=False)
    xt = nc.declare_dram_parameter("xt", [T, NR], f32, isOutput=False)
    xhl = nc.declare_dram_parameter("xhl", [128, NR, 2], f16, isOutput=False)
    bout = nc.declare_dram_parameter("bout", [T, NR], f32, isOutput=False)
    out = nc.declare_dram_parameter("out", [T, NR], f32, isOutput=True)

    with tile.TileContext(nc) as tc:
        with (
            tc.tile_pool(name="const", bufs=1) as constp,
            tc.tile_pool(name="wabc", bufs=2) as wabc,
            tc.tile_pool(name="wop", bufs=3) as wop,
            tc.tile_pool(name="ohp", bufs=2) as ohp,
            tc.tile_pool(name="front", bufs=3) as frontp,
            tc.tile_pool(name="backp", bufs=2) as backp,
            tc.tile_pool(name="acc", bufs=1) as accp,
            tc.tile_pool(name="psxc", bufs=2, space="PSUM") as psxcp,
        ):
            xt_sb = constp.tile([T, NR], f32)
            nc.scalar.dma_start(xt_sb[:], xt[:])
            xhl_sb = constp.tile([128, NR * 2], f16)
            nc.scalar.dma_start(xhl_sb[:], xhl[:])
            bo_sb = constp.tile([T, NR], f32)
            nc.scalar.dma_start(bo_sb[:], bout[:])

            red = accp.tile([T, NR], f32)

            state = {}

            def front(s):
                n0 = s * G
                nsl = slice(n0, n0 + G)
                w1a = wabc.tile([128, G * H], f32, tag="w1a")
                nc.sync.dma_start(w1a[:], w1all[:, nsl])
                ba = wabc.tile([128, G * H], f32, tag="ba")
                nc.sync.dma_start(ba[:], ball[:, nsl])
                w0a = wabc.tile([128, G * H], f32, tag="w0a")
                nc.sync.dma_start(w0a[:], w0all[:, nsl])
                woa = wop.tile([128, G * H], f32, tag="woa")
                nc.sync.dma_start(woa[:], woall[:, nsl])
                oh = ohp.tile([128, G * T], f8, tag="oh")
                nc.scalar.dma_start(oh[:], ohall[:, n0 * T : (n0 + G) * T])

                xc2_ps = psxcp.tile([128, 2 * G], f32, tag="xc")
                for g in range(G):
                    n = n0 + g
                    nc.tensor.matmul(
                        xc2_ps[:, g : g + 1],
                        oh[:, g * T : (g + 1) * T],
                        xhl_sb[:, 2 * n : 2 * n + 1],
                        start=True,
                        stop=True,
                    )
                    nc.tensor.matmul(
                        xc2_ps[:, G + g : G + g + 1],
                        oh[:, g * T : (g + 1) * T],
                        xhl_sb[:, 2 * n + 1 : 2 * n + 2],
                        start=True,
                        stop=True,
                    )
                xc2_sb = frontp.tile([128, 2 * G], f32, tag="xc2sb")
                nc.scalar.copy(xc2_sb[:], xc2_ps[:])
                xc_sb = frontp.tile([128, G], f32, tag="xcsb")
                nc.vector.scalar_tensor_tensor(
                    xc_sb[:],
                    xc2_sb[:, G : 2 * G],
                    1.0 / LO_SCALE,
                    xc2_sb[:, 0:G],
                    Alu.mult,
                    Alu.add,
                )

                m1 = frontp.tile([128, G * H], f32, tag="m1")
                nc.vector.tensor_tensor(
                    m1[:].rearrange("p (g h) -> p g h", g=G),
                    w1a[:].rearrange("p (g h) -> p g h", g=G),
                    xt_sb[:, nsl].broadcast_to([128, G, H]),
                    Alu.mult,
                )
                v = frontp.tile([128, G * H], f32, tag="v")
                nc.gpsimd.tensor_tensor(v[:], m1[:], ba[:], Alu.add)

                a0 = frontp.tile([128, G * H], f32, tag="a0")
                for g in range(G):
                    nc.scalar.activation(
                        a0[:, g * H : (g + 1) * H],
                        w0a[:, g * H : (g + 1) * H],
                        Act.Copy,
                        scale=xc_sb[:, g : g + 1],
                    )
                state[s] = (nsl, v, a0, woa)

            def back(s):
                nsl, v, a0, woa = state.pop(s)
                u = backp.tile([128, G * H], f32, tag="u")
                ueng = nc.vector if s % 2 == 0 else nc.gpsimd
                ueng.tensor_tensor(u[:], a0[:], v[:], Alu.add)
                r = backp.tile([128, G * H], f32, tag="r")
                nc.vector.scalar_tensor_tensor(
                    r[:], u[:], 0.0, woa[:], Alu.max, Alu.mult
                )
                nc.vector.tensor_reduce(
                    red[:, nsl],
                    r[:].rearrange("p (g h) -> p g h", g=G),
                    axis=mybir.AxisListType.X,
                    op=Alu.add,
                )

            for stage in range(S + 1):
                if stage < S:
                    front(stage)
                if stage >= 1:
                    back(stage - 1)

            lg = backp.tile([T, NR], f32, tag="lg")
            nc.vector.tensor_tensor(lg[:], red[:], bo_sb[:], Alu.add)
            ot = backp.tile([T, NR], f32, tag="ot")
            nc.scalar.activation(ot[:], lg[:], Act.Sigmoid)
            nc.sync.dma_start(out[:], ot[:])

    nc.compile()
    _cache["nc"] = nc
    return nc


def make_in_maps(X, W_in, b_in, W_out, b_out, col_idx):
    X = np.asarray(X, dtype=np.float32)
    W_in = np.asarray(W_in, dtype=np.float32)
    b_in = np.asarray(b_in, dtype=np.float32)
    W_out = np.asarray(W_out, dtype=np.float32)
    b_out = np.asarray(b_out, dtype=np.float32)
    col_idx = np.asarray(col_idx)

    jj = np.arange(128)
    in_maps = []
    for c in range(M):
        sl = slice(c * NR, (c + 1) * NR)
        Wc = W_in[sl]  # [NR, T, H, 2]
        w0all = np.ascontiguousarray(Wc[:, :, :, 0].transpose(1, 0, 2))
        w1all = np.ascontiguousarray(Wc[:, :, :, 1].transpose(1, 0, 2))
        ball = np.ascontiguousarray(b_in[sl].transpose(1, 0, 2))
        woall = np.ascontiguousarray(W_out[sl].transpose(1, 0, 2))

        ohall = (col_idx[sl].reshape(1, -1) == jj[:, None]).astype(
            ml_dtypes.float8_e4m3
        )

        xtc = np.ascontiguousarray(X[sl].T)  # [t, n] f32
        xhi = xtc.astype(np.float16)
        xlo = ((xtc - xhi.astype(np.float32)) * LO_SCALE).astype(np.float16)
        xhl = np.stack([xhi, xlo], axis=-1)  # [128, NR, 2]

        in_maps.append(
            {
                "w0all": w0all,
                "w1all": w1all,
                "ball": ball,
                "woall": woall,
                "ohall": ohall,
                "xt": xtc,
                "xhl": xhl,
                "bout": np.ascontiguousarray(b_out[sl].T),
            }
        )
    return in_maps


def kernel(X, W_in, b_in, W_out, b_out, col_idx):
    from concourse.bass_utils import run_bass_kernel_spmd

    nc = _build()
    in_maps = make_in_maps(X, W_in, b_in, W_out, b_out, col_idx)
    res = run_bass_kernel_spmd(nc, in_maps, list(range(M))).results
    out = np.empty((N, T), np.float32)
    for c in range(M):
        out[c * NR : (c + 1) * NR] = res[c]["out"].T
    return out



# revision 3
# speedup vs baseline: 1.7478x; 1.7478x over previous
"""Trainium2 Bass kernel for the per-cell-MLP "MAR one-sided missingness" model.

Model (per cell (n,t) of a 1024x128 grid):
    xc     = X[n, col_idx[n,t]]
    h      = relu(W_in[n,t,:,0]*xc + W_in[n,t,:,1]*X[n,t] + b_in[n,t,:])   # [H]
    out    = sigmoid(dot(W_out[n,t,:], h) + b_out[n,t])

Sharding: rows N split across 8 cores (128 rows each), fully data parallel.

The kernel is HBM-bandwidth bound: the four per-cell weight tensors dominate
traffic, so they stream as float16 (rel-err ~1e-2 vs the 2e-2 gate, checked
empirically), halving bytes vs f32: 16 MB/core. The neighbor gather
X[n, col_idx[n,t]] is a pure indexing operation and is staged on the host
(like the baseline's host-built one-hot masks, minus the on-device matmul).

Per-core layout: partition dim = t, free dims = (h, n) with n innermost so
the per-cell scalars x[t,n], xc[t,n] broadcast over h via a stride-0 MIDDLE
AP dim — keeping every DVE tensor_tensor in 2x_1p f16 mode (the mode check
only looks at the innermost dim). Streaming over h in blocks of HB=16:

  DVE : m1 = w1 * x_bc          (f16 TT, 2x)
  DVE : a0 = w0 * xc_bc         (f16 TT, 2x)
  PE  : u  = I@m1 + I@a0 + I@b  (identity matmuls accumulate in PSUM f32)
  ACT : ru = relu(u)            (PSUM->SBUF copy with fused ReLU, f16 out)
  DVE : r  = ru * wo            (f16 TT, 2x)
  Pool/DVE: fold r over h by contiguous halves (h outer => halves are
            contiguous slices), last fold + block-accumulate in f32.
Epilogue: out = sigmoid(acc + b_out^T), DMA out, host transposes back.
"""

import numpy as np

N, T, H = 1024, 128, 128
M = 8            # cores
NR = N // M      # rows per core
HB = 16          # h-block size
NB = H // HB     # 8 h-blocks
FD = HB * NR     # free elems per block

_cache = {}


def _build():
    if "nc" in _cache:
        return _cache["nc"]
    import concourse.bacc as bacc
    import concourse.mybir as mybir
    import concourse.tile as tile

    f32 = mybir.dt.float32
    f16 = mybir.dt.float16
    Alu = mybir.AluOpType
    Act = mybir.ActivationFunctionType

    nc = bacc.Bacc()
    w0 = nc.declare_dram_parameter("w0", [T, H * NR], f16, isOutput=False)
    w1 = nc.declare_dram_parameter("w1", [T, H * NR], f16, isOutput=False)
    bb = nc.declare_dram_parameter("bb", [T, H * NR], f16, isOutput=False)
    wo = nc.declare_dram_parameter("wo", [T, H * NR], f16, isOutput=False)
    xt = nc.declare_dram_parameter("xt", [T, NR], f16, isOutput=False)
    xct = nc.declare_dram_parameter("xct", [T, NR], f16, isOutput=False)
    bout = nc.declare_dram_parameter("bout", [T, NR], f32, isOutput=False)
    ident = nc.declare_dram_parameter("ident", [128, 128], f16, isOutput=False)
    out = nc.declare_dram_parameter("out", [T, NR], f32, isOutput=True)

    with tile.TileContext(nc) as tc:
        with (
            tc.tile_pool(name="const", bufs=1) as constp,
            tc.tile_pool(name="w", bufs=2) as wp,
            tc.tile_pool(name="mid", bufs=2) as midp,
            tc.tile_pool(name="ps", bufs=2, space="PSUM") as psp,
        ):
            xt_sb = constp.tile([T, NR], f16)
            nc.scalar.dma_start(xt_sb[:], xt[:])
            xct_sb = constp.tile([T, NR], f16)
            nc.scalar.dma_start(xct_sb[:], xct[:])
            bo_sb = constp.tile([T, NR], f32)
            nc.scalar.dma_start(bo_sb[:], bout[:])
            id_sb = constp.tile([128, 128], f16)
            nc.scalar.dma_start(id_sb[:], ident[:])
            acc = constp.tile([T, NR], f32)

            xb = (
                xt_sb[:].rearrange("p (o n) -> p o n", o=1).broadcast_to([T, HB, NR])
            )
            xcb = (
                xct_sb[:].rearrange("p (o n) -> p o n", o=1).broadcast_to([T, HB, NR])
            )

            for s in range(NB):
                csl = slice(s * FD, (s + 1) * FD)
                w1t = wp.tile([T, FD], f16, tag="w1")
                nc.sync.dma_start(w1t[:], w1[:, csl])
                w0t = wp.tile([T, FD], f16, tag="w0")
                nc.sync.dma_start(w0t[:], w0[:, csl])
                bbt = wp.tile([T, FD], f16, tag="bb")
                nc.sync.dma_start(bbt[:], bb[:, csl])
                wot = wp.tile([T, FD], f16, tag="wo")
                nc.sync.dma_start(wot[:], wo[:, csl])

                m1 = midp.tile([T, FD], f16, tag="m1")
                nc.vector.tensor_tensor(
                    m1[:].rearrange("p (h n) -> p h n", h=HB),
                    w1t[:].rearrange("p (h n) -> p h n", h=HB),
                    xb,
                    Alu.mult,
                )
                a0 = midp.tile([T, FD], f16, tag="a0")
                nc.vector.tensor_tensor(
                    a0[:].rearrange("p (h n) -> p h n", h=HB),
                    w0t[:].rearrange("p (h n) -> p h n", h=HB),
                    xcb,
                    Alu.mult,
                )

                ups = psp.tile([T, FD], f32, tag="u")
                for q in range(FD // 512):
                    qs = slice(q * 512, (q + 1) * 512)
                    nc.tensor.matmul(
                        ups[:, qs], id_sb[:], m1[:, qs], start=True, stop=False
                    )
                    nc.tensor.matmul(
                        ups[:, qs], id_sb[:], a0[:, qs], start=False, stop=False
                    )
                    nc.tensor.matmul(
                        ups[:, qs], id_sb[:], bbt[:, qs], start=False, stop=True
                    )

                ru = midp.tile([T, FD], f16, tag="ru")
                nc.scalar.activation(ru[:], ups[:], Act.Relu)

                r = midp.tile([T, FD], f16, tag="r")
                nc.vector.tensor_tensor(r[:], ru[:], wot[:], Alu.mult)

                # reduce over h: halves are contiguous since h is the outer
                # free dim. Two biggest folds on Pool, rest on DVE; final
                # fold + block accumulation in f32.
                f1 = midp.tile([T, FD // 2], f16, tag="f1")
                nc.gpsimd.tensor_tensor(f1[:], r[:, : FD // 2], r[:, FD // 2 :], Alu.add)
                f2 = midp.tile([T, FD // 4], f16, tag="f2")
                nc.gpsimd.tensor_tensor(f2[:], f1[:, : FD // 4], f1[:, FD // 4 :], Alu.add)
                f3 = midp.tile([T, FD // 8], f16, tag="f3")
                nc.vector.tensor_tensor(f3[:], f2[:, : FD // 8], f2[:, FD // 8 :], Alu.add)
                if s == 0:
                    nc.vector.tensor_tensor(acc[:], f3[:, :NR], f3[:, NR:], Alu.add)
                else:
                    rb = midp.tile([T, NR], f32, tag="rb")
                    nc.vector.tensor_tensor(rb[:], f3[:, :NR], f3[:, NR:], Alu.add)
                    nc.vector.tensor_tensor(acc[:], acc[:], rb[:], Alu.add)

            lg = midp.tile([T, NR], f32, tag="lg")
            nc.vector.tensor_tensor(lg[:], acc[:], bo_sb[:], Alu.add)
            ot = midp.tile([T, NR], f32, tag="ot")
            nc.scalar.activation(ot[:], lg[:], Act.Sigmoid)
            nc.sync.dma_start(out[:], ot[:])

    nc.compile()
    _cache["nc"] = nc
    return nc


def make_in_maps(X, W_in, b_in, W_out, b_out, col_idx):
    X = np.asarray(X, dtype=np.float32)
    W_in = np.asarray(W_in, dtype=np.float32)
    b_in = np.asarray(b_in, dtype=np.float32)
    W_out = np.asarray(W_out, dtype=np.float32)
    b_out = np.asarray(b_out, dtype=np.float32)
    col_idx = np.asarray(col_idx)

    xc = np.take_along_axis(X, col_idx, axis=1)  # [N, T] neighbor gather
    ident = np.eye(128, dtype=np.float16)

    w0g = W_in[:, :, :, 0].astype(np.float16)  # [N, T, H]
    w1g = W_in[:, :, :, 1].astype(np.float16)
    bbg = b_in.astype(np.float16)
    wog = W_out.astype(np.float16)

    in_maps = []
    for c in range(M):
        sl = slice(c * NR, (c + 1) * NR)

        def t_hn(a):  # [NR, T, H] -> [T, H*NR] f16 contiguous
            return np.ascontiguousarray(a[sl].transpose(1, 2, 0)).reshape(T, H * NR)

        in_maps.append(
            {
                "w0": t_hn(w0g),
                "w1": t_hn(w1g),
                "bb": t_hn(bbg),
                "wo": t_hn(wog),
                "xt": np.ascontiguousarray(X[sl].T.astype(np.float16)),
                "xct": np.ascontiguousarray(xc[sl].T.astype(np.float16)),
                "bout": np.ascontiguousarray(b_out[sl].T),
                "ident": ident,
            }
        )
    return in_maps


def kernel(X, W_in, b_in, W_out, b_out, col_idx):
    from concourse.bass_utils import run_bass_kernel_spmd

    nc = _build()
    in_maps = make_in_maps(X, W_in, b_in, W_out, b_out, col_idx)
    res = run_bass_kernel_spmd(nc, in_maps, list(range(M))).results
    out = np.empty((N, T), np.float32)
    for c in range(M):
        out[c * NR : (c + 1) * NR] = res[c]["out"].T
    return out
